# revision 2
# baseline (speedup 1.0000x reference)
"""Trainium2 Bass kernel for nn_Net_23210003267823 (BiGCN rumor-detection net).

Math (per branch, edge set A, weights W1,b1,W2,b2):
    U  = x @ W1                                  (big GEMM, memory-bound: x is 400 MB)
    Y  = D^-1/2 U ;  h1 = D^-1/2 (A Y + Y) + b1  (sym-normalized GCN conv w/ self loops)
    Q  = relu(x[root]) @ W2[64:]                 (root-extend: 128 distinct root rows, replicated)
    z  = relu(h1) @ W2[:64] + Q[batch]
    h2 = relu(D^-1/2 (A Zt + Zt) + b2),  Zt = D^-1/2 z
    out_branch = [segment_mean(h2, batch) | h1[root] * (cnt>0)]
Final: log_softmax(concat(td, bu) @ fc_W + fc_b).

Sharding: nodes row-sharded over 8 cores (2500 real + 60 pad rows each).
Chunked AllGather of the 128-wide bf16 message tables overlaps the producing
phase; aggregation via one merged dma_gather per dst-block (both branches) +
is_equal one-hot matmuls into PSUM. Q[batch], root-gather and segment-sum are
one-hot matmuls from SBUF-resident tables (no DRAM round trips).
Host prep is integer index metadata only (edge partition/sort, degree counts).
"""
import sys, os
sys.path.insert(0, "/opt/trn_rl_repo")
import numpy as np

NC_ = 8
N, E, G = 20000, 320000, 128
IN, HID, OUT = 5000, 64, 64
RPC, PRC, NBLK = 2500, 2560, 20   # real rows/core, padded rows/core, row blocks
NPAD = NC_ * PRC                   # 20480
INP, NK = 5120, 40                 # padded IN, K blocks
NCH = 5                            # AllGather chunks per table
CBLK = NBLK // NCH                 # dst blocks per chunk
BIG = np.float32(1e30)

_cache = {}


def _wrap16(idx):
    """dma_gather wrapped-index layout: [128, n/16] i16, idx i at (p = i%16 (replicated), c = i//16)."""
    n = idx.shape[-1]
    out = np.zeros(idx.shape[:-1] + (128, n // 16), np.int16)
    cols = np.arange(n // 16)
    for p in range(128):
        out[..., p, :] = idx[..., cols * 16 + (p % 16)]
    return out


def _build(TB):
    KSTOP = int(os.environ.get("KSTOP", "99"))
    import concourse.bass as bass
    import concourse.mybir as mybir
    import concourse.tile as tile
    from concourse import bacc, library_config

    dt = mybir.dt
    f32, bf16, i32, i16 = dt.float32, dt.bfloat16, dt.int32, dt.int16
    AF = mybir.ActivationFunctionType
    OP = mybir.AluOpType

    nc = bacc.Bacc("TRN2", target_bir_lowering=False, debug=False, num_devices=NC_)

    TB2 = 2 * TB
    NE = TB2 * 128  # merged (both-branch) gathered rows per dst block

    # ---------------- I/O ----------------
    xc = nc.dram_tensor("xc", [RPC, IN], f32, kind="ExternalInput")
    xroot = nc.dram_tensor("xroot", [G, IN], f32, kind="ExternalInput")
    w1 = nc.dram_tensor("w1", [IN, 128], f32, kind="ExternalInput")
    w2a = nc.dram_tensor("w2a", [128, 128], f32, kind="ExternalInput")
    w2b = nc.dram_tensor("w2b", [IN, 128], f32, kind="ExternalInput")
    bias1 = nc.dram_tensor("bias1", [128, 128], f32, kind="ExternalInput")
    bias2 = nc.dram_tensor("bias2", [128, 128], f32, kind="ExternalInput")
    deg = nc.dram_tensor("deg", [2, PRC], f32, kind="ExternalInput")
    srcs = nc.dram_tensor("srcs", [NBLK, 128, TB2 * 8], i16, kind="ExternalInput")
    drel = nc.dram_tensor("drel", [NBLK, 128, TB2], f32, kind="ExternalInput")
    brel = nc.dram_tensor("brel", [PRC], f32, kind="ExternalInput")      # batch id per local row, [p b] layout via rearrange
    browb = nc.dram_tensor("browb", [128, PRC], f32, kind="ExternalInput")  # batch id per local row, bcast over partitions
    rshb = nc.dram_tensor("rshb", [128, NBLK * G], f32, kind="ExternalInput")  # rloc[g]-blk*128, bcast over partitions
    cntf = nc.dram_tensor("cntf", [128, 1], f32, kind="ExternalInput")   # graph sizes
    iota_in = nc.dram_tensor("iota_in", [128, 128], f32, kind="ExternalInput")
    iotac_in = nc.dram_tensor("iotac_in", [128, 1], f32, kind="ExternalInput")
    fcw = nc.dram_tensor("fcw", [2, 128, 256], f32, kind="ExternalInput")
    fcb = nc.dram_tensor("fcb", [128, 2], f32, kind="ExternalInput")
    out = nc.dram_tensor("out", [G, 2], f32, kind="ExternalOutput")

    # ---------------- internal DRAM ----------------
    Ytl = nc.dram_tensor("Ytl", [PRC, 128], bf16)
    Ytf = nc.dram_tensor("Ytf", [NPAD, 128], bf16, addr_space="Shared")
    Ztl = nc.dram_tensor("Ztl", [PRC, 128], bf16)
    Ztf = nc.dram_tensor("Ztf", [NPAD, 128], bf16, addr_space="Shared")
    arl = nc.dram_tensor("arl", [128, 256], f32)
    arf = nc.dram_tensor("arf", [128, 256], f32, addr_space="Shared")

    RG = [list(range(NC_))]
    Ytf_v = Ytf[:].rearrange("(c r) f -> c r f", c=NC_)
    Ztf_v = Ztf[:].rearrange("(c r) f -> c r f", c=NC_)
    CROWS = PRC // NCH  # 512 rows per collective chunk

    with tile.TileContext(nc) as tc:
        with tc.tile_pool(name="const", bufs=1) as cp:
            nc.gpsimd.load_library(library_config.mlp)

            iob = cp.tile([128, 128], bf16)
            nc.sync.dma_start(out=iob[:], in_=iota_in[:])
            iocb = cp.tile([128, 1], bf16)
            nc.sync.dma_start(out=iocb[:], in_=iotac_in[:])

            # dinv [128, 40]: col br*NBLK+blk
            dga = cp.tile([128, NBLK * 2], f32)
            nc.sync.dma_start(out=dga[:], in_=deg[:].rearrange("t (b p) -> p (t b)", p=128))
            drc = cp.tile([128, NBLK * 2], f32)
            nc.vector.reciprocal(drc[:], dga[:])
            dinv = cp.tile([128, NBLK * 2], f32)
            nc.scalar.activation(dinv[:], drc[:], AF.Sqrt)

            b1t = cp.tile([128, 128], f32)
            nc.sync.dma_start(out=b1t[:], in_=bias1[:])
            b2t = cp.tile([128, 128], f32)
            nc.sync.dma_start(out=b2t[:], in_=bias2[:])
            w2at = cp.tile([128, 128], bf16)
            nc.sync.dma_start(out=w2at[:], in_=w2a[:])
            brelt = cp.tile([128, NBLK], bf16)
            nc.sync.dma_start(out=brelt[:], in_=brel[:].rearrange("(b p) -> p b", p=128))
            fcw0 = cp.tile([128, 256], f32)
            nc.sync.dma_start(out=fcw0[:], in_=fcw[0])
            fcw1 = cp.tile([128, 256], f32)
            nc.sync.dma_start(out=fcw1[:], in_=fcw[1])
            fcbt = cp.tile([128, 2], f32)
            nc.sync.dma_start(out=fcbt[:], in_=fcb[:])
            cnt = cp.tile([128, 1], f32)
            nc.sync.dma_start(out=cnt[:], in_=cntf[:])

            # one-hot tables (bf16, built once)
            browt = cp.tile([128, NBLK, 128], bf16)
            nc.scalar.dma_start(out=browt[:], in_=browb[:].rearrange("p (b r) -> p b r", r=128))
            rsht = cp.tile([128, NBLK, 128], bf16)
            nc.scalar.dma_start(out=rsht[:], in_=rshb[:].rearrange("p (b g) -> p b g", g=128))
            boh_g = cp.tile([128, NBLK, 128], bf16)  # [g, blk, r] = (batch[r] == g)
            nc.vector.tensor_tensor(out=boh_g[:],
                                    in0=iocb[:, :, None].to_broadcast([128, NBLK, 128]),
                                    in1=browt[:], op=OP.is_equal)
            boh_r = cp.tile([128, NBLK, 128], bf16)  # [r, blk, g] = (batch[r] == g)
            nc.vector.tensor_tensor(out=boh_r[:],
                                    in0=brelt[:, :, None].to_broadcast([128, NBLK, 128]),
                                    in1=iob[:, None, :].to_broadcast([128, NBLK, 128]),
                                    op=OP.is_equal)
            roh = cp.tile([128, NBLK, 128], bf16)    # [r, blk, g] = (rloc[g] == blk*128+r)
            nc.vector.tensor_tensor(out=roh[:],
                                    in0=iocb[:, :, None].to_broadcast([128, NBLK, 128]),
                                    in1=rsht[:], op=OP.is_equal)

            # persistent SBUF feature tables
            Ysb = cp.tile([128, NBLK, 128], bf16)   # Y = dinv * U, [r, blk, f]
            Zsb = cp.tile([128, NBLK, 128], bf16)   # Zt = dinv * z
            h1sb = cp.tile([128, NBLK, 128], bf16)  # h1 (pre-relu)
            qb = cp.tile([128, 128], bf16)          # Q rows

            # ---------------- phase G: U = x @ W1 ; Y ; chunked AllGather; Q ----------------
            if KSTOP >= 1:
             with tc.tile_pool(name="pw", bufs=1) as pw, \
                 tc.tile_pool(name="px", bufs=5) as px, \
                 tc.tile_pool(name="pxt", bufs=2) as pxt, \
                 tc.tile_pool(name="pub", bufs=3) as pub, \
                 tc.tile_pool(name="pup", bufs=2, space="PSUM") as pup, \
                 tc.tile_pool(name="pqp", bufs=1, space="PSUM") as pqp:
                w1all = pw.tile([128, NK * 128], bf16)
                nc.vector.memset(w1all[:, 39 * 128:], 0.0)
                nc.gpsimd.dma_start(out=w1all[:, 0:39 * 128].rearrange("p (k f) -> p k f", f=128),
                                    in_=w1[0:4992, :].rearrange("(k p) f -> p k f", p=128))
                nc.gpsimd.dma_start(out=w1all[0:8, 39 * 128:40 * 128], in_=w1[4992:IN, :])

                for rc in range(NCH):
                    xbs = []
                    for j in range(4):
                        bi = rc * 4 + j
                        row0 = bi * 128
                        nr = min(128, RPC - row0)
                        xb = px.tile([128, INP], bf16, tag="xb")
                        if nr < 128:
                            nc.vector.memset(xb[:], 0.0)
                        else:
                            nc.vector.memset(xb[:, IN:INP], 0.0)
                        nc.scalar.dma_start(out=xb[0:nr, 0:IN], in_=xc[row0:row0 + nr, :])
                        xbs.append(xb)
                    pu = pup.tile([128, 512], f32)
                    xtc = pxt.tile([128, NK, 4, 128], bf16, tag="xtc")
                    for j in range(4):
                        nc.sync.dma_start(out=xtc[:, :, j, :], in_=xbs[j][:], transpose=True)
                    for k in range(NK):
                        nc.tensor.matmul(out=pu[:], lhsT=w1all[:, k * 128:(k + 1) * 128], rhs=xtc[:, k, :, :],
                                         start=(k == 0), stop=(k == NK - 1))
                    ut = pub.tile([128, 512], bf16, tag="ut")
                    nc.vector.tensor_copy(ut[:], pu[:])
                    ubt = pub.tile([128, 4, 128], bf16, tag="ubt")
                    nc.sync.dma_start(out=ubt[:], in_=ut[:], transpose=True)
                    for j in range(4):
                        bi = rc * 4 + j
                        nc.vector.tensor_scalar(out=Ysb[:, bi, 0:64], in0=ubt[:, j, 0:64],
                                                scalar1=dinv[:, bi:bi + 1], scalar2=None, op0=OP.mult)
                        nc.vector.tensor_scalar(out=Ysb[:, bi, 64:128], in0=ubt[:, j, 64:128],
                                                scalar1=dinv[:, NBLK + bi:NBLK + bi + 1], scalar2=None, op0=OP.mult)
                        nc.sync.dma_start(out=Ytl[bi * 128:(bi + 1) * 128, :], in_=Ysb[:, bi, :])
                    # chunked AllGather: rows [rc*512, (rc+1)*512) are final
                    nc.gpsimd.collective_compute(
                        "AllGather", OP.bypass, replica_groups=RG,
                        ins=[Ytl[rc * CROWS:(rc + 1) * CROWS, :]],
                        outs=[Ytf_v[:, rc * CROWS:(rc + 1) * CROWS, :]])

                # ---- root rows -> Q (replicated on every core) ----
                xrt = px.tile([128, INP], f32, tag="xb2")
                nc.vector.memset(xrt[:, IN:INP], 0.0)
                nc.scalar.dma_start(out=xrt[:, 0:IN], in_=xroot[:])
                xrr = px.tile([128, INP], bf16, tag="xb3")
                nc.scalar.activation(xrr[:], xrt[:], AF.Relu)
                w2ball = pw.tile([128, NK * 128], bf16)
                nc.vector.memset(w2ball[:, 39 * 128:], 0.0)
                nc.gpsimd.dma_start(out=w2ball[:, 0:39 * 128].rearrange("p (k f) -> p k f", f=128),
                                    in_=w2b[0:4992, :].rearrange("(k p) f -> p k f", p=128))
                nc.gpsimd.dma_start(out=w2ball[0:8, 39 * 128:40 * 128], in_=w2b[4992:IN, :])
                rtall = pxt.tile([128, NK, 128], bf16, tag="rtall")
                nc.sync.dma_start(out=rtall[:], in_=xrr[:], transpose=True)
                pq = pqp.tile([128, 128], f32)
                for k in range(NK):
                    nc.tensor.matmul(out=pq[:], lhsT=rtall[:, k, :], rhs=w2ball[:, k * 128:(k + 1) * 128],
                                     start=(k == 0), stop=(k == NK - 1))
                nc.vector.tensor_copy(qb[:], pq[:])

            # ---------------- conv helper: merged both-branch aggregation ----------------
            def conv_block(pools, table, blk):
                """Returns (oh, V) tiles for dst block blk; matmuls done by caller."""
                pa, pv, po = pools
                st = pa.tile([128, TB2 * 8], i16, tag="st")
                nc.sync.dma_start(out=st[:], in_=srcs[blk])
                dr_ = pa.tile([128, TB2], bf16, tag="dr")
                nc.sync.dma_start(out=dr_[:], in_=drel[blk])
                V = pv.tile([128, TB2, 128], bf16, tag="v")
                nc.gpsimd.dma_gather(V[:], table[:], st[:], NE, NE, 128, single_packet=False)
                oh = po.tile([128, TB2, 128], bf16, tag="oh")
                nc.vector.tensor_tensor(out=oh[:],
                                        in0=dr_[:, :, None].to_broadcast([128, TB2, 128]),
                                        in1=iob[:, None, :].to_broadcast([128, TB2, 128]),
                                        op=OP.is_equal)
                return oh, V

            # ---------------- phase C1: conv1 -> h1, z, Zt; chunked AllGather Z ----------------
            if KSTOP >= 2:
             with tc.tile_pool(name="pa1", bufs=4) as pa, \
                 tc.tile_pool(name="pv1", bufs=4) as pv, \
                 tc.tile_pool(name="po1", bufs=4) as po, \
                 tc.tile_pool(name="pm1", bufs=3) as pm, \
                 tc.tile_pool(name="ph1", bufs=3, space="PSUM") as ph, \
                 tc.tile_pool(name="pz1", bufs=2, space="PSUM") as pz:
                pools = (pa, pv, po)
                for blk in range(NBLK):
                    oh, V = conv_block(pools, Ytf, blk)
                    for br in range(2):
                        ph_ = ph.tile([128, 64], f32)
                        for t in range(TB):
                            tt = br * TB + t
                            nc.tensor.matmul(out=ph_[:], lhsT=oh[:, tt, :], rhs=V[:, tt, br * 64:(br + 1) * 64],
                                             start=(t == 0), stop=(t == TB - 1))
                        hs = pm.tile([128, 64], f32, tag="hs")
                        nc.vector.tensor_tensor(out=hs[:], in0=ph_[:], in1=Ysb[:, blk, br * 64:(br + 1) * 64],
                                                op=OP.add)
                        nc.vector.tensor_scalar(out=hs[:], in0=hs[:],
                                                scalar1=dinv[:, br * NBLK + blk:br * NBLK + blk + 1],
                                                scalar2=None, op0=OP.mult)
                        nc.vector.tensor_tensor(out=h1sb[:, blk, br * 64:(br + 1) * 64], in0=hs[:],
                                                in1=b1t[:, br * 64:(br + 1) * 64], op=OP.add)
                    hr = pm.tile([128, 128], bf16, tag="hr")
                    nc.scalar.activation(hr[:], h1sb[:, blk, :], AF.Relu)
                    hrT = pm.tile([128, 128], bf16, tag="hrT")
                    nc.sync.dma_start(out=hrT[:], in_=hr[:], transpose=True)
                    pz_ = pz.tile([128, 128], f32)
                    nc.tensor.matmul(out=pz_[:], lhsT=hrT[:], rhs=w2at[:], start=True, stop=False)
                    nc.tensor.matmul(out=pz_[:], lhsT=boh_g[:, blk, :], rhs=qb[:], start=False, stop=True)
                    nc.vector.tensor_scalar(out=Zsb[:, blk, 0:64], in0=pz_[:, 0:64],
                                            scalar1=dinv[:, blk:blk + 1], scalar2=None, op0=OP.mult)
                    nc.vector.tensor_scalar(out=Zsb[:, blk, 64:128], in0=pz_[:, 64:128],
                                            scalar1=dinv[:, NBLK + blk:NBLK + blk + 1], scalar2=None, op0=OP.mult)
                    nc.sync.dma_start(out=Ztl[blk * 128:(blk + 1) * 128, :], in_=Zsb[:, blk, :])
                    if blk % CBLK == CBLK - 1:
                        rc = blk // CBLK
                        nc.gpsimd.collective_compute(
                            "AllGather", OP.bypass, replica_groups=RG,
                            ins=[Ztl[rc * CROWS:(rc + 1) * CROWS, :]],
                            outs=[Ztf_v[:, rc * CROWS:(rc + 1) * CROWS, :]])

            # ---------------- phase C2: conv2 -> h2 -> segment sums ----------------
            if KSTOP >= 3:
             with tc.tile_pool(name="pa2", bufs=4) as pa2, \
                 tc.tile_pool(name="pv2", bufs=4) as pv2, \
                 tc.tile_pool(name="po2", bufs=4) as po2, \
                 tc.tile_pool(name="pm2", bufs=3) as pm2, \
                 tc.tile_pool(name="ph2", bufs=3, space="PSUM") as ph2, \
                 tc.tile_pool(name="ps2", bufs=1, space="PSUM") as ps2:
                pools2 = (pa2, pv2, po2)
                pseg = ps2.tile([128, 128], f32)
                prg = ps2.tile([128, 128], f32)
                for blk in range(NBLK):
                    oh, V = conv_block(pools2, Ztf, blk)
                    pay = pm2.tile([128, 128], bf16, tag="pay")
                    for br in range(2):
                        ph_ = ph2.tile([128, 64], f32)
                        for t in range(TB):
                            tt = br * TB + t
                            nc.tensor.matmul(out=ph_[:], lhsT=oh[:, tt, :], rhs=V[:, tt, br * 64:(br + 1) * 64],
                                             start=(t == 0), stop=(t == TB - 1))
                        hs2 = pm2.tile([128, 64], f32, tag="hs2")
                        nc.vector.tensor_tensor(out=hs2[:], in0=ph_[:], in1=Zsb[:, blk, br * 64:(br + 1) * 64],
                                                op=OP.add)
                        nc.vector.tensor_scalar(out=hs2[:], in0=hs2[:],
                                                scalar1=dinv[:, br * NBLK + blk:br * NBLK + blk + 1],
                                                scalar2=None, op0=OP.mult)
                        nc.vector.tensor_tensor(out=hs2[:], in0=hs2[:],
                                                in1=b2t[:, br * 64:(br + 1) * 64], op=OP.add)
                        nc.scalar.activation(pay[:, br * 64:(br + 1) * 64], hs2[:], AF.Relu)
                    nc.tensor.matmul(out=pseg[:], lhsT=boh_r[:, blk, :], rhs=pay[:],
                                     start=(blk == 0), stop=(blk == NBLK - 1))
                    nc.tensor.matmul(out=prg[:], lhsT=roh[:, blk, :], rhs=h1sb[:, blk, :],
                                     start=(blk == 0), stop=(blk == NBLK - 1))

                part = pm2.tile([128, 256], f32, tag="part")
                nc.vector.tensor_copy(part[:, 0:128], pseg[:])
                nc.vector.tensor_copy(part[:, 128:256], prg[:])
                nc.sync.dma_start(out=arl[:], in_=part[:])

            if KSTOP >= 4:
             nc.gpsimd.collective_compute("AllReduce", OP.add, replica_groups=RG,
                                          ins=[arl[:]], outs=[arf[:]])

            # ---------------- final ----------------
            if KSTOP >= 4:
             with tc.tile_pool(name="pf", bufs=1) as pf:
                Rt = pf.tile([128, 256], f32)
                nc.sync.dma_start(out=Rt[:], in_=arf[:])
                c1 = pf.tile([128, 1], f32)
                nc.vector.tensor_scalar_max(out=c1[:], in0=cnt[:], scalar1=1.0)
                rec = pf.tile([128, 1], f32)
                nc.vector.reciprocal(rec[:], c1[:])
                ind = pf.tile([128, 1], f32)
                nc.vector.tensor_scalar_min(out=ind[:], in0=cnt[:], scalar1=1.0)
                hfc = pf.tile([128, 256], f32)
                nc.vector.tensor_scalar(out=hfc[:, 0:64], in0=Rt[:, 0:64], scalar1=rec[:, :1], scalar2=None, op0=OP.mult)
                nc.vector.tensor_scalar(out=hfc[:, 64:128], in0=Rt[:, 128:192], scalar1=ind[:, :1], scalar2=None, op0=OP.mult)
                nc.vector.tensor_scalar(out=hfc[:, 128:192], in0=Rt[:, 64:128], scalar1=rec[:, :1], scalar2=None, op0=OP.mult)
                nc.vector.tensor_scalar(out=hfc[:, 192:256], in0=Rt[:, 192:256], scalar1=ind[:, :1], scalar2=None, op0=OP.mult)
                lg = pf.tile([128, 2], f32)
                for j, fw in enumerate((fcw0, fcw1)):
                    tmp = pf.tile([128, 256], f32, tag=f"tmp{j}")
                    nc.vector.tensor_tensor(out=tmp[:], in0=hfc[:], in1=fw[:], op=OP.mult)
                    nc.vector.reduce_sum(lg[:, j:j + 1], tmp[:], axis=mybir.AxisListType.X)
                nc.vector.tensor_tensor(out=lg[:], in0=lg[:], in1=fcbt[:], op=OP.add)
                mx = pf.tile([128, 1], f32)
                nc.vector.reduce_max(mx[:], lg[:], axis=mybir.AxisListType.X)
                d_ = pf.tile([128, 2], f32)
                nc.vector.tensor_scalar(out=d_[:], in0=lg[:], scalar1=mx[:, :1], scalar2=None, op0=OP.subtract)
                e_ = pf.tile([128, 2], f32)
                nc.scalar.activation(e_[:], d_[:], AF.Exp)
                s_ = pf.tile([128, 1], f32)
                nc.vector.reduce_sum(s_[:], e_[:], axis=mybir.AxisListType.X)
                ls = pf.tile([128, 1], f32)
                nc.scalar.activation(ls[:], s_[:], AF.Ln)
                ov = pf.tile([128, 2], f32)
                nc.vector.tensor_scalar(out=ov[:], in0=d_[:], scalar1=ls[:, :1], scalar2=None, op0=OP.subtract)
                nc.sync.dma_start(out=out[:], in_=ov[:])

    nc.compile()
    return nc


def _prep(x, edge_index, bu_edge_index, batch, root_index,
          W1_td, b1_td, W2_td, b2_td, W1_bu, b1_bu, W2_bu, b2_bu, fc_W, fc_b):
    """Host-side: integer index metadata + parameter reshaping (no float math on data)."""
    x = np.asarray(x, np.float32)
    batch = np.asarray(batch).astype(np.int64)
    root_index = np.asarray(root_index).astype(np.int64)
    edges = [np.asarray(edge_index).astype(np.int64), np.asarray(bu_edge_index).astype(np.int64)]

    degs = []
    for ei in edges:
        d = np.bincount(ei[1], minlength=N).astype(np.int64) + 1
        degs.append(d)

    maxcnt = 0
    blk_edges = [[[None] * NBLK for _ in range(2)] for _ in range(NC_)]
    for br, ei in enumerate(edges):
        src, dst = ei[0], ei[1]
        c = dst // RPC
        loc = dst - c * RPC
        blk = loc // 128
        rel = loc - blk * 128
        ps = (src // RPC) * PRC + (src - (src // RPC) * RPC)
        key = c * NBLK + blk
        order = np.argsort(key, kind="stable")
        ks = key[order]
        bounds = np.searchsorted(ks, np.arange(NC_ * NBLK + 1))
        for c_ in range(NC_):
            for b_ in range(NBLK):
                sl = order[bounds[c_ * NBLK + b_]:bounds[c_ * NBLK + b_ + 1]]
                blk_edges[c_][br][b_] = (ps[sl], rel[sl])
                maxcnt = max(maxcnt, len(sl))
    TB = max(1, (maxcnt + 127) // 128)
    TB2 = 2 * TB

    # merged both-branch index/dst-rel tables: td slots [0, TB*128), bu slots [TB*128, 2*TB*128)
    srcs_flat = np.zeros((NC_, NBLK, TB2 * 128), np.int64)
    drel = np.full((NC_, NBLK, 128, TB2), -1.0, np.float32)
    for c in range(NC_):
        for b in range(NBLK):
            for br in range(2):
                s, r = blk_edges[c][br][b]
                n = len(s)
                off = br * TB * 128
                srcs_flat[c, b, off:off + n] = s
                lane, til = np.arange(n) % 128, np.arange(n) // 128
                drel[c, b, lane, br * TB + til] = r
    srcs16 = _wrap16(srcs_flat.reshape(NC_ * NBLK, TB2 * 128)).reshape(NC_, NBLK, 128, TB2 * 8)

    deg = np.full((NC_, 2, PRC), BIG, np.float32)
    for br in range(2):
        deg[:, br, :RPC] = degs[br].reshape(NC_, RPC).astype(np.float32)

    brel = np.full((NC_, PRC), -1.0, np.float32)
    brel[:, :RPC] = batch.reshape(NC_, RPC).astype(np.float32)
    browb = np.broadcast_to(brel[:, None, :], (NC_, 128, PRC)).copy()

    # rloc[g]: local row of root g on its owning core, else out-of-range marker
    rc = root_index // RPC
    rl = root_index - rc * RPC
    rsh = np.full((NC_, NBLK, G), 1e9, np.float32)
    for g in range(G):
        for b in range(NBLK):
            rsh[rc[g], b, g] = rl[g] - b * 128
    rshb = np.broadcast_to(rsh.reshape(NC_, 1, NBLK * G), (NC_, 128, NBLK * G)).copy()

    cnt = np.bincount(batch, minlength=G).astype(np.float32)
    cntf = np.broadcast_to(cnt.reshape(G, 1), (G, 1)).copy()

    # parameters (pure reshapes / replication)
    w1 = np.hstack([np.asarray(W1_td, np.float32), np.asarray(W1_bu, np.float32)])        # [5000,128]
    w2a = np.zeros((128, 128), np.float32)  # block-diag: one K=128 matmul covers both branches
    w2a[0:64, 0:64] = np.asarray(W2_td, np.float32)[:HID]
    w2a[64:128, 64:128] = np.asarray(W2_bu, np.float32)[:HID]
    w2b = np.hstack([np.asarray(W2_td, np.float32)[HID:], np.asarray(W2_bu, np.float32)[HID:]])  # [5000,128]
    bias1 = np.broadcast_to(np.concatenate([np.asarray(b1_td, np.float32), np.asarray(b1_bu, np.float32)]), (128, 128)).copy()
    bias2 = np.broadcast_to(np.concatenate([np.asarray(b2_td, np.float32), np.asarray(b2_bu, np.float32)]), (128, 128)).copy()
    fcw = np.stack([np.broadcast_to(np.asarray(fc_W, np.float32)[:, j], (128, 256)) for j in range(2)])
    fcb = np.broadcast_to(np.asarray(fc_b, np.float32), (128, 2)).copy()
    iota_in = np.tile(np.arange(128, dtype=np.float32), (128, 1))
    iotac_in = np.arange(128, dtype=np.float32).reshape(128, 1)
    xroot = np.ascontiguousarray(x[root_index])

    in_maps = []
    for c in range(NC_):
        in_maps.append(dict(
            xc=np.ascontiguousarray(x[c * RPC:(c + 1) * RPC]),
            xroot=xroot,
            w1=w1, w2a=w2a, w2b=w2b, bias1=bias1, bias2=bias2,
            deg=np.ascontiguousarray(deg[c]),
            srcs=np.ascontiguousarray(srcs16[c]), drel=np.ascontiguousarray(drel[c]),
            brel=np.ascontiguousarray(brel[c]), browb=np.ascontiguousarray(browb[c]),
            rshb=np.ascontiguousarray(rshb[c]), cntf=cntf,
            iota_in=iota_in, iotac_in=iotac_in, fcw=np.ascontiguousarray(fcw), fcb=fcb,
        ))
    return TB, in_maps


def kernel(**inputs):
    from concourse.bass_utils import run_bass_kernel_spmd
    TB, in_maps = _prep(**inputs)
    if TB not in _cache:
        _cache[TB] = _build(TB)
    nc = _cache[TB]
    res = run_bass_kernel_spmd(nc, in_maps, list(range(NC_)))
    return res.results[0]["out"]


if __name__ == "__main__":
    import reference
    inputs = {k: np.asarray(v) for k, v in reference.setup_inputs().items()}
    got = kernel(**inputs)
    print(got[:4])


# revision 38
# speedup vs baseline: 1.2619x; 1.2619x over previous
"""Trainium2 Bass kernel for nn_Net_23210003267823 (BiGCN rumor-detection net).

Math (per branch, edge set A, weights W1,b1,W2,b2):
    U  = x @ W1                                  (big GEMM, memory-bound: x is 400 MB)
    Y  = D^-1/2 U ;  h1 = D^-1/2 (A Y + Y) + b1  (sym-normalized GCN conv w/ self loops)
    Q  = relu(x[root]) @ W2[64:]                 (root-extend: 128 distinct root rows, replicated)
    z  = relu(h1) @ W2[:64] + Q[batch]
    h2 = relu(D^-1/2 (A Zt + Zt) + b2),  Zt = D^-1/2 z
    out_branch = [segment_mean(h2, batch) | h1[root] * (cnt>0)]
Final: log_softmax(concat(td, bu) @ fc_W + fc_b).

Sharding: nodes row-sharded over 8 cores (2500 real + 60 pad rows each).
Chunked AllGather of the 128-wide bf16 message tables overlaps the producing
phase; aggregation via one merged dma_gather per dst-block (both branches) +
is_equal one-hot matmuls into PSUM. Q[batch], root-gather and segment-sum are
one-hot matmuls from SBUF-resident tables (no DRAM round trips).
Host prep is integer index metadata only (edge partition/sort, degree counts).
"""
import sys, os
sys.path.insert(0, "/opt/trn_rl_repo")
import numpy as np

NC_ = 8
N, E, G = 20000, 320000, 128
IN, HID, OUT = 5000, 64, 64
RPC, PRC, NBLK = 2500, 2560, 20   # real rows/core, padded rows/core, row blocks
NPAD = NC_ * PRC                   # 20480
INP, NK = 5120, 40                 # padded IN, K blocks
NCH = 5                            # AllGather chunks per table
CBLK = NBLK // NCH                 # dst blocks per chunk
BIG = np.float32(1e30)

_cache = {}


def _wrap16(idx):
    """dma_gather wrapped-index layout: [128, n/16] i16, idx i at (p = i%16 (replicated), c = i//16)."""
    n = idx.shape[-1]
    out = np.zeros(idx.shape[:-1] + (128, n // 16), np.int16)
    cols = np.arange(n // 16)
    for p in range(128):
        out[..., p, :] = idx[..., cols * 16 + (p % 16)]
    return out


def _build(TB):
    KSTOP = int(os.environ.get("KSTOP", "99"))
    import concourse.bass as bass
    import concourse.mybir as mybir
    import concourse.tile as tile
    from concourse import bacc, library_config

    dt = mybir.dt
    f32, bf16, i32, i16 = dt.float32, dt.bfloat16, dt.int32, dt.int16
    AF = mybir.ActivationFunctionType
    OP = mybir.AluOpType

    nc = bacc.Bacc("TRN2", target_bir_lowering=False, debug=False, num_devices=NC_)

    TB2 = 2 * TB
    NE = TB2 * 128  # merged (both-branch) gathered rows per dst block

    # ---------------- I/O ----------------
    xcT = nc.dram_tensor("xcT", [IN, RPC], f32, kind="ExternalInput")
    xrootT = nc.dram_tensor("xrootT", [IN, G], f32, kind="ExternalInput")
    w1 = nc.dram_tensor("w1", [IN, 128], f32, kind="ExternalInput")
    w2a = nc.dram_tensor("w2a", [128, 128], f32, kind="ExternalInput")
    w2b = nc.dram_tensor("w2b", [IN, 128], f32, kind="ExternalInput")
    bias1 = nc.dram_tensor("bias1", [128, 128], f32, kind="ExternalInput")
    bias2 = nc.dram_tensor("bias2", [128, 128], f32, kind="ExternalInput")
    deg = nc.dram_tensor("deg", [2, PRC], f32, kind="ExternalInput")
    srcsY = nc.dram_tensor("srcsY", [NBLK, 128, TB2 * 8], i16, kind="ExternalInput")
    srcsZ = nc.dram_tensor("srcsZ", [NBLK, 128, TB2 * 8], i16, kind="ExternalInput")
    drel = nc.dram_tensor("drel", [NBLK, 128, TB2], f32, kind="ExternalInput")
    brel = nc.dram_tensor("brel", [PRC], f32, kind="ExternalInput")      # batch id per local row, [p b] layout via rearrange
    browb = nc.dram_tensor("browb", [128, PRC], f32, kind="ExternalInput")  # batch id per local row, bcast over partitions
    rshb = nc.dram_tensor("rshb", [128, NBLK * G], f32, kind="ExternalInput")  # rloc[g]-blk*128, bcast over partitions
    cntf = nc.dram_tensor("cntf", [128, 1], f32, kind="ExternalInput")   # graph sizes
    iota_in = nc.dram_tensor("iota_in", [128, 128], f32, kind="ExternalInput")
    iotac_in = nc.dram_tensor("iotac_in", [128, 1], f32, kind="ExternalInput")
    fcw = nc.dram_tensor("fcw", [2, 128, 256], f32, kind="ExternalInput")
    fcb = nc.dram_tensor("fcb", [128, 2], f32, kind="ExternalInput")
    out = nc.dram_tensor("out", [G, 2], f32, kind="ExternalOutput")

    # ---------------- internal DRAM ----------------
    Ytl = nc.dram_tensor("Ytl", [PRC, 128], bf16)
    Ytf = nc.dram_tensor("Ytf", [NPAD, 128], bf16, addr_space="Shared")
    Ztl = nc.dram_tensor("Ztl", [PRC, 128], bf16)
    Ztf = nc.dram_tensor("Ztf", [NPAD, 128], bf16, addr_space="Shared")
    arl = nc.dram_tensor("arl", [128, 256], f32)
    arf = nc.dram_tensor("arf", [128, 256], f32, addr_space="Shared")

    RG = [list(range(NC_))]
    HROWS = PRC // 2  # rows per collective half-chunk

    with tile.TileContext(nc) as tc:
        with tc.tile_pool(name="const", bufs=1) as cp:
            nc.gpsimd.load_library(library_config.mlp)

            def ag_half(dst_l, dst_f, h):
                # chunk-major layout: half h of the local table lands in the
                # contiguous range [h*NC_*HROWS, (h+1)*NC_*HROWS) of dst_f
                nc.gpsimd.collective_compute(
                    "AllGather", OP.bypass, replica_groups=RG,
                    ins=[dst_l[h * HROWS:(h + 1) * HROWS, :]],
                    outs=[dst_f[h * NC_ * HROWS:(h + 1) * NC_ * HROWS, :]])

            iob = cp.tile([128, 128], bf16)
            nc.gpsimd.dma_start(out=iob[:], in_=iota_in[:])
            iocb = cp.tile([128, 1], bf16)
            nc.gpsimd.dma_start(out=iocb[:], in_=iotac_in[:])

            # dinv [128, 40]: col br*NBLK+blk
            dga = cp.tile([128, NBLK * 2], f32)
            nc.sync.dma_start(out=dga[:], in_=deg[:].rearrange("t (b p) -> p (t b)", p=128))
            drc = cp.tile([128, NBLK * 2], f32)
            nc.vector.reciprocal(drc[:], dga[:])
            dinv = cp.tile([128, NBLK * 2], f32)
            nc.scalar.activation(dinv[:], drc[:], AF.Sqrt)

            b1t = cp.tile([128, 128], f32)
            nc.sync.dma_start(out=b1t[:], in_=bias1[:])
            b2t = cp.tile([128, 128], f32)
            nc.sync.dma_start(out=b2t[:], in_=bias2[:])
            w2at = cp.tile([128, 128], bf16)
            nc.gpsimd.dma_start(out=w2at[:], in_=w2a[:])
            brelt = cp.tile([128, NBLK], bf16)
            nc.gpsimd.dma_start(out=brelt[:], in_=brel[:].rearrange("(b p) -> p b", p=128))
            fcbt = cp.tile([128, 2], f32)
            nc.sync.dma_start(out=fcbt[:], in_=fcb[:])
            cnt = cp.tile([128, 1], f32)
            nc.sync.dma_start(out=cnt[:], in_=cntf[:])

            # one-hot tables (bf16; loads/builds emitted later to keep the
            # gpsimd queue free for the x stream)
            boh_g = cp.tile([128, NBLK, 128], bf16)  # [g, blk, r] = (batch[r] == g)
            boh_r = cp.tile([128, NBLK, 128], bf16)  # [r, blk, g] = (batch[r] == g)
            roh = cp.tile([128, NBLK, 128], bf16)    # [r, blk, g] = (rloc[g] == blk*128+r)

            def build_onehots(pool):
                browt = pool.tile([128, NBLK, 128], bf16, tag="browt")
                nc.gpsimd.dma_start(out=browt[:], in_=browb[:].rearrange("p (b r) -> p b r", r=128))
                rsht = pool.tile([128, NBLK, 128], bf16, tag="rsht")
                nc.gpsimd.dma_start(out=rsht[:], in_=rshb[:].rearrange("p (b g) -> p b g", g=128))
                nc.vector.tensor_tensor(out=boh_g[:],
                                        in0=iocb[:, :, None].to_broadcast([128, NBLK, 128]),
                                        in1=browt[:], op=OP.is_equal)
                nc.vector.tensor_tensor(out=boh_r[:],
                                        in0=brelt[:, :, None].to_broadcast([128, NBLK, 128]),
                                        in1=iob[:, None, :].to_broadcast([128, NBLK, 128]),
                                        op=OP.is_equal)
                nc.vector.tensor_tensor(out=roh[:],
                                        in0=iocb[:, :, None].to_broadcast([128, NBLK, 128]),
                                        in1=rsht[:], op=OP.is_equal)

            # persistent SBUF feature tables
            Ysb = cp.tile([128, NBLK, 128], bf16)   # Y = dinv * U, [r, blk, f]
            Zsb = cp.tile([128, NBLK, 128], bf16)   # Zt = dinv * z
            h1sb = cp.tile([128, NBLK, 128], bf16)  # h1 (pre-relu)
            qb = cp.tile([128, 128], bf16)          # Q rows

            # ---------------- phase G: U = x @ W1 ; Y ; chunked AllGather; Q ----------------
            if KSTOP >= 1:
             with tc.tile_pool(name="pw", bufs=1) as pw, \
                 tc.tile_pool(name="ppn", bufs=4) as ppn, \
                 tc.tile_pool(name="prt", bufs=1) as prt, \
                 tc.tile_pool(name="pub", bufs=2) as pub, \
                 tc.tile_pool(name="pup", bufs=1, space="PSUM") as pup, \
                 tc.tile_pool(name="pqp", bufs=1, space="PSUM") as pqp:
                w1all = pw.tile([128, NK * 128], bf16, tag="wall")
                nc.vector.memset(w1all[:, 39 * 128:], 0.0)
                nc.gpsimd.dma_start(out=w1all[:, 0:39 * 128].rearrange("p (k f) -> p k f", f=128),
                                    in_=w1[0:4992, :].rearrange("(k p) f -> p k f", p=128))
                nc.gpsimd.dma_start(out=w1all[0:8, 39 * 128:40 * 128], in_=w1[4992:IN, :])

                # warm-zero the panel ring slots once (stale data stays finite)
                for _ in range(4):
                    wz = ppn.tile([128, PRC], bf16, tag="pk")
                    nc.vector.memset(wz[:], 0.0)

                # U^T accumulated in 5 persistent PSUM banks; x^T panels stream
                # straight from DRAM (host-transposed) with no on-chip transpose
                pus = []
                for rc in range(NCH):
                    pu_rc = pup.tile([128, 512], f32, tag=f"pu{rc}", name=f"pu{rc}")
                    pus.append(pu_rc)
                for k in range(NK):
                    pk = ppn.tile([128, PRC], bf16, tag="pk")
                    if k < 39:
                        nc.gpsimd.dma_start(out=pk[:, 0:RPC], in_=xcT[k * 128:(k + 1) * 128, :])
                    else:
                        nc.gpsimd.dma_start(out=pk[0:8, 0:RPC], in_=xcT[4992:IN, :])
                    for rc in range(NCH):
                        nc.tensor.matmul(out=pus[rc][:], lhsT=w1all[:, k * 128:(k + 1) * 128],
                                         rhs=pk[:, rc * 512:(rc + 1) * 512],
                                         start=(k == 0), stop=(k == NK - 1))
                for rc in range(NCH):
                    ut = pub.tile([128, 512], bf16, tag="ut")
                    nc.vector.tensor_copy(ut[:], pus[rc][:])
                    ubt = pub.tile([128, 4, 128], bf16, tag="ubt")
                    nc.scalar.dma_start(out=ubt[:], in_=ut[:], transpose=True)
                    for j in range(4):
                        bi = rc * 4 + j
                        nc.vector.tensor_scalar(out=Ysb[:, bi, 0:64], in0=ubt[:, j, 0:64],
                                                scalar1=dinv[:, bi:bi + 1], scalar2=None, op0=OP.mult)
                        nc.vector.tensor_scalar(out=Ysb[:, bi, 64:128], in0=ubt[:, j, 64:128],
                                                scalar1=dinv[:, NBLK + bi:NBLK + bi + 1], scalar2=None, op0=OP.mult)
                        nc.scalar.dma_start(out=Ytl[bi * 128:(bi + 1) * 128, :], in_=Ysb[:, bi, :])

                # ---- root rows -> Q (replicated on every core) ----
                rtp = prt.tile([128, NK * 128], bf16, tag="rtp")
                nc.vector.memset(rtp[:, 39 * 128:], 0.0)
                nc.gpsimd.dma_start(out=rtp[:, 0:39 * 128].rearrange("p (k g) -> p k g", g=128),
                                    in_=xrootT[0:4992, :].rearrange("(k p) g -> p k g", p=128))
                nc.gpsimd.dma_start(out=rtp[0:8, 39 * 128:40 * 128], in_=xrootT[4992:IN, :])
                rtr = prt.tile([128, NK * 128], bf16, tag="rtr")
                nc.scalar.activation(rtr[:], rtp[:], AF.Relu)
                w2ball = pw.tile([128, NK * 128], bf16, tag="wall")
                nc.vector.memset(w2ball[:, 39 * 128:], 0.0)
                nc.gpsimd.dma_start(out=w2ball[:, 0:39 * 128].rearrange("p (k f) -> p k f", f=128),
                                    in_=w2b[0:4992, :].rearrange("(k p) f -> p k f", p=128))
                nc.gpsimd.dma_start(out=w2ball[0:8, 39 * 128:40 * 128], in_=w2b[4992:IN, :])
                build_onehots(prt)
                # single Y AllGather (all rows finish at once; core-major layout)
                nc.gpsimd.collective_compute(
                    "AllGather", OP.bypass, replica_groups=RG,
                    ins=[Ytl[:]], outs=[Ytf[:]])
                pq = pqp.tile([128, 128], f32)
                for k in range(NK):
                    nc.tensor.matmul(out=pq[:], lhsT=rtr[:, k * 128:(k + 1) * 128],
                                     rhs=w2ball[:, k * 128:(k + 1) * 128],
                                     start=(k == 0), stop=(k == NK - 1))
                nc.vector.tensor_copy(qb[:], pq[:])

            # ---------------- conv helper: merged both-branch aggregation ----------------
            def conv_block(pools, table, srcs_t, blk):
                """Returns (oh, V) tiles for dst block blk; matmuls done by caller."""
                pa, pv, po = pools
                st = pa.tile([128, TB2 * 8], i16, tag="st")
                nc.sync.dma_start(out=st[:], in_=srcs_t[blk])
                dr_ = pa.tile([128, TB2], bf16, tag="dr")
                nc.gpsimd.dma_start(out=dr_[:], in_=drel[blk])
                V = pv.tile([128, TB2, 128], bf16, tag="v")
                nc.gpsimd.dma_gather(V[:], table[:], st[:], NE, NE, 128, single_packet=False)
                oh = po.tile([128, TB2, 128], bf16, tag="oh")
                nc.vector.tensor_tensor(out=oh[:],
                                        in0=dr_[:, :, None].to_broadcast([128, TB2, 128]),
                                        in1=iob[:, None, :].to_broadcast([128, TB2, 128]),
                                        op=OP.is_equal)
                return oh, V

            # ---------------- phase C1: conv1 -> h1, z, Zt; chunked AllGather Z ----------------
            if KSTOP >= 2:
             with tc.tile_pool(name="pa1", bufs=6) as pa, \
                 tc.tile_pool(name="pv1", bufs=7) as pv, \
                 tc.tile_pool(name="po1", bufs=7) as po, \
                 tc.tile_pool(name="pm1", bufs=6) as pm, \
                 tc.tile_pool(name="ph1", bufs=4, space="PSUM") as ph, \
                 tc.tile_pool(name="pz1", bufs=3, space="PSUM") as pz:
                pools = (pa, pv, po)

                def z_stage(blk, hr):
                    """z = relu(h1) @ W2a + Q[batch]; Zt = dinv*z; spill chunk rows."""
                    hrT = pm.tile([128, 128], bf16, tag="hrT")
                    nc.scalar.dma_start(out=hrT[:], in_=hr[:], transpose=True)
                    pz_ = pz.tile([128, 128], f32)
                    nc.tensor.matmul(out=pz_[:], lhsT=hrT[:], rhs=w2at[:], start=True, stop=False)
                    nc.tensor.matmul(out=pz_[:], lhsT=boh_g[:, blk, :], rhs=qb[:], start=False, stop=True)
                    nc.vector.tensor_scalar(out=Zsb[:, blk, 0:64], in0=pz_[:, 0:64],
                                            scalar1=dinv[:, blk:blk + 1], scalar2=None, op0=OP.mult)
                    nc.vector.tensor_scalar(out=Zsb[:, blk, 64:128], in0=pz_[:, 64:128],
                                            scalar1=dinv[:, NBLK + blk:NBLK + blk + 1], scalar2=None, op0=OP.mult)
                    nc.scalar.dma_start(out=Ztl[blk * 128:(blk + 1) * 128, :], in_=Zsb[:, blk, :])

                zq = []
                for blk in range(NBLK):
                    # half-table AllGather emitted once blocks 0-9 have spilled
                    if blk == 13:
                        ag_half(Ztl, Ztf, 0)
                    oh, V = conv_block(pools, Ytf, srcsY, blk)
                    for br in range(2):
                        ph_ = ph.tile([128, 64], f32)
                        for t in range(TB):
                            tt = br * TB + t
                            nc.tensor.matmul(out=ph_[:], lhsT=oh[:, tt, :], rhs=V[:, tt, br * 64:(br + 1) * 64],
                                             start=(t == 0), stop=(t == TB - 1))
                        hs = pm.tile([128, 64], f32, tag="hs")
                        nc.vector.tensor_tensor(out=hs[:], in0=ph_[:], in1=Ysb[:, blk, br * 64:(br + 1) * 64],
                                                op=OP.add)
                        nc.vector.tensor_scalar(out=hs[:], in0=hs[:],
                                                scalar1=dinv[:, br * NBLK + blk:br * NBLK + blk + 1],
                                                scalar2=None, op0=OP.mult)
                        nc.vector.tensor_tensor(out=h1sb[:, blk, br * 64:(br + 1) * 64], in0=hs[:],
                                                in1=b1t[:, br * 64:(br + 1) * 64], op=OP.add)
                    hr = pm.tile([128, 128], bf16, tag="hr")
                    nc.scalar.activation(hr[:], h1sb[:, blk, :], AF.Relu)
                    # z stage delayed three blocks: its transpose/relu chain is
                    # ready by then, so the PE queue never stalls between block aggs
                    zq.append((blk, hr))
                    if len(zq) > 3:
                        z_stage(*zq.pop(0))
                for item in zq:
                    z_stage(*item)
                ag_half(Ztl, Ztf, 1)

            # ---------------- phase C2: conv2 -> h2 -> segment sums ----------------
            if KSTOP >= 3:
             with tc.tile_pool(name="pa2", bufs=6) as pa2, \
                 tc.tile_pool(name="pv2", bufs=7) as pv2, \
                 tc.tile_pool(name="po2", bufs=7) as po2, \
                 tc.tile_pool(name="pm2", bufs=3) as pm2, \
                 tc.tile_pool(name="ph2", bufs=4, space="PSUM") as ph2, \
                 tc.tile_pool(name="ps2", bufs=1, space="PSUM") as ps2:
                pools2 = (pa2, pv2, po2)
                pseg = ps2.tile([128, 128], f32)
                prg = ps2.tile([128, 128], f32)

                def seg_stage(blk, pay):
                    nc.tensor.matmul(out=pseg[:], lhsT=boh_r[:, blk, :], rhs=pay[:],
                                     start=(blk == 0), stop=(blk == NBLK - 1))
                    nc.tensor.matmul(out=prg[:], lhsT=roh[:, blk, :], rhs=h1sb[:, blk, :],
                                     start=(blk == 0), stop=(blk == NBLK - 1))

                prev2 = None
                for blk in range(NBLK):
                    oh, V = conv_block(pools2, Ztf, srcsZ, blk)
                    pay = pm2.tile([128, 128], bf16, tag="pay")
                    for br in range(2):
                        ph_ = ph2.tile([128, 64], f32)
                        for t in range(TB):
                            tt = br * TB + t
                            nc.tensor.matmul(out=ph_[:], lhsT=oh[:, tt, :], rhs=V[:, tt, br * 64:(br + 1) * 64],
                                             start=(t == 0), stop=(t == TB - 1))
                        hs2 = pm2.tile([128, 64], f32, tag="hs2")
                        nc.vector.tensor_tensor(out=hs2[:], in0=ph_[:], in1=Zsb[:, blk, br * 64:(br + 1) * 64],
                                                op=OP.add)
                        nc.vector.tensor_scalar(out=hs2[:], in0=hs2[:],
                                                scalar1=dinv[:, br * NBLK + blk:br * NBLK + blk + 1],
                                                scalar2=None, op0=OP.mult)
                        nc.vector.tensor_tensor(out=hs2[:], in0=hs2[:],
                                                in1=b2t[:, br * 64:(br + 1) * 64], op=OP.add)
                        nc.scalar.activation(pay[:, br * 64:(br + 1) * 64], hs2[:], AF.Relu)
                    if prev2 is not None:
                        seg_stage(*prev2)
                    prev2 = (blk, pay)
                seg_stage(*prev2)

                part = pm2.tile([128, 256], f32, tag="part")
                nc.vector.tensor_copy(part[:, 0:128], pseg[:])
                nc.vector.tensor_copy(part[:, 128:256], prg[:])
                nc.sync.dma_start(out=arl[:], in_=part[:])

            if KSTOP >= 4:
             nc.gpsimd.collective_compute("AllReduce", OP.add, replica_groups=RG,
                                          ins=[arl[:]], outs=[arf[:]])

            # ---------------- final ----------------
            if KSTOP >= 4:
             with tc.tile_pool(name="pf", bufs=1) as pf:
                fcw0 = pf.tile([128, 256], f32, tag="fcw0")
                nc.sync.dma_start(out=fcw0[:], in_=fcw[0])
                fcw1 = pf.tile([128, 256], f32, tag="fcw1")
                nc.sync.dma_start(out=fcw1[:], in_=fcw[1])
                Rt = pf.tile([128, 256], f32)
                nc.sync.dma_start(out=Rt[:], in_=arf[:])
                c1 = pf.tile([128, 1], f32)
                nc.vector.tensor_scalar_max(out=c1[:], in0=cnt[:], scalar1=1.0)
                rec = pf.tile([128, 1], f32)
                nc.vector.reciprocal(rec[:], c1[:])
                ind = pf.tile([128, 1], f32)
                nc.vector.tensor_scalar_min(out=ind[:], in0=cnt[:], scalar1=1.0)
                hfc = pf.tile([128, 256], f32)
                nc.vector.tensor_scalar(out=hfc[:, 0:64], in0=Rt[:, 0:64], scalar1=rec[:, :1], scalar2=None, op0=OP.mult)
                nc.vector.tensor_scalar(out=hfc[:, 64:128], in0=Rt[:, 128:192], scalar1=ind[:, :1], scalar2=None, op0=OP.mult)
                nc.vector.tensor_scalar(out=hfc[:, 128:192], in0=Rt[:, 64:128], scalar1=rec[:, :1], scalar2=None, op0=OP.mult)
                nc.vector.tensor_scalar(out=hfc[:, 192:256], in0=Rt[:, 192:256], scalar1=ind[:, :1], scalar2=None, op0=OP.mult)
                lg = pf.tile([128, 2], f32)
                for j, fw in enumerate((fcw0, fcw1)):
                    tmp = pf.tile([128, 256], f32, tag=f"tmp{j}")
                    nc.vector.tensor_tensor(out=tmp[:], in0=hfc[:], in1=fw[:], op=OP.mult)
                    nc.vector.reduce_sum(lg[:, j:j + 1], tmp[:], axis=mybir.AxisListType.X)
                nc.vector.tensor_tensor(out=lg[:], in0=lg[:], in1=fcbt[:], op=OP.add)
                mx = pf.tile([128, 1], f32)
                nc.vector.reduce_max(mx[:], lg[:], axis=mybir.AxisListType.X)
                d_ = pf.tile([128, 2], f32)
                nc.vector.tensor_scalar(out=d_[:], in0=lg[:], scalar1=mx[:, :1], scalar2=None, op0=OP.subtract)
                e_ = pf.tile([128, 2], f32)
                nc.scalar.activation(e_[:], d_[:], AF.Exp)
                s_ = pf.tile([128, 1], f32)
                nc.vector.reduce_sum(s_[:], e_[:], axis=mybir.AxisListType.X)
                ls = pf.tile([128, 1], f32)
                nc.scalar.activation(ls[:], s_[:], AF.Ln)
                ov = pf.tile([128, 2], f32)
                nc.vector.tensor_scalar(out=ov[:], in0=d_[:], scalar1=ls[:, :1], scalar2=None, op0=OP.subtract)
                nc.sync.dma_start(out=out[:], in_=ov[:])

    nc.compile()
    return nc


def _prep(x, edge_index, bu_edge_index, batch, root_index,
          W1_td, b1_td, W2_td, b2_td, W1_bu, b1_bu, W2_bu, b2_bu, fc_W, fc_b):
    """Host-side: integer index metadata + parameter reshaping (no float math on data)."""
    x = np.asarray(x, np.float32)
    batch = np.asarray(batch).astype(np.int64)
    root_index = np.asarray(root_index).astype(np.int64)
    edges = [np.asarray(edge_index).astype(np.int64), np.asarray(bu_edge_index).astype(np.int64)]

    degs = []
    for ei in edges:
        d = np.bincount(ei[1], minlength=N).astype(np.int64) + 1
        degs.append(d)

    maxcnt = 0
    blk_edges = [[[None] * NBLK for _ in range(2)] for _ in range(NC_)]
    for br, ei in enumerate(edges):
        src, dst = ei[0], ei[1]
        c = dst // RPC
        loc = dst - c * RPC
        blk = loc // 128
        rel = loc - blk * 128
        ps = (src // RPC) * PRC + (src - (src // RPC) * RPC)
        key = c * NBLK + blk
        order = np.argsort(key, kind="stable")
        ks = key[order]
        bounds = np.searchsorted(ks, np.arange(NC_ * NBLK + 1))
        for c_ in range(NC_):
            for b_ in range(NBLK):
                sl = order[bounds[c_ * NBLK + b_]:bounds[c_ * NBLK + b_ + 1]]
                blk_edges[c_][br][b_] = (ps[sl], rel[sl])
                maxcnt = max(maxcnt, len(sl))
    TB = max(1, (maxcnt + 127) // 128)
    TB2 = 2 * TB

    # merged both-branch index/dst-rel tables: td slots [0, TB*128), bu slots [TB*128, 2*TB*128)
    srcs_flat = np.zeros((NC_, NBLK, TB2 * 128), np.int64)
    drel = np.full((NC_, NBLK, 128, TB2), -1.0, np.float32)
    for c in range(NC_):
        for b in range(NBLK):
            for br in range(2):
                s, r = blk_edges[c][br][b]
                n = len(s)
                off = br * TB * 128
                srcs_flat[c, b, off:off + n] = s
                lane, til = np.arange(n) % 128, np.arange(n) // 128
                drel[c, b, lane, br * TB + til] = r
    srcs16 = _wrap16(srcs_flat.reshape(NC_ * NBLK, TB2 * 128)).reshape(NC_, NBLK, 128, TB2 * 8)
    # chunk-major encoding for the Z table (two contiguous AllGather halves)
    HR = PRC // 2
    sc, sl = srcs_flat // PRC, srcs_flat % PRC
    srcsZ_flat = (sl // HR) * (NC_ * HR) + sc * HR + sl % HR
    srcsZ16 = _wrap16(srcsZ_flat.reshape(NC_ * NBLK, TB2 * 128)).reshape(NC_, NBLK, 128, TB2 * 8)

    deg = np.full((NC_, 2, PRC), BIG, np.float32)
    for br in range(2):
        deg[:, br, :RPC] = degs[br].reshape(NC_, RPC).astype(np.float32)

    brel = np.full((NC_, PRC), -1.0, np.float32)
    brel[:, :RPC] = batch.reshape(NC_, RPC).astype(np.float32)
    browb = np.broadcast_to(brel[:, None, :], (NC_, 128, PRC)).copy()

    # rloc[g]: local row of root g on its owning core, else out-of-range marker
    rc = root_index // RPC
    rl = root_index - rc * RPC
    rsh = np.full((NC_, NBLK, G), 1e9, np.float32)
    for g in range(G):
        for b in range(NBLK):
            rsh[rc[g], b, g] = rl[g] - b * 128
    rshb = np.broadcast_to(rsh.reshape(NC_, 1, NBLK * G), (NC_, 128, NBLK * G)).copy()

    cnt = np.bincount(batch, minlength=G).astype(np.float32)
    cntf = np.broadcast_to(cnt.reshape(G, 1), (G, 1)).copy()

    # parameters (pure reshapes / replication)
    w1 = np.hstack([np.asarray(W1_td, np.float32), np.asarray(W1_bu, np.float32)])        # [5000,128]
    w2a = np.zeros((128, 128), np.float32)  # block-diag: one K=128 matmul covers both branches
    w2a[0:64, 0:64] = np.asarray(W2_td, np.float32)[:HID]
    w2a[64:128, 64:128] = np.asarray(W2_bu, np.float32)[:HID]
    w2b = np.hstack([np.asarray(W2_td, np.float32)[HID:], np.asarray(W2_bu, np.float32)[HID:]])  # [5000,128]
    bias1 = np.broadcast_to(np.concatenate([np.asarray(b1_td, np.float32), np.asarray(b1_bu, np.float32)]), (128, 128)).copy()
    bias2 = np.broadcast_to(np.concatenate([np.asarray(b2_td, np.float32), np.asarray(b2_bu, np.float32)]), (128, 128)).copy()
    fcw = np.stack([np.broadcast_to(np.asarray(fc_W, np.float32)[:, j], (128, 256)) for j in range(2)])
    fcb = np.broadcast_to(np.asarray(fc_b, np.float32), (128, 2)).copy()
    iota_in = np.tile(np.arange(128, dtype=np.float32), (128, 1))
    iotac_in = np.arange(128, dtype=np.float32).reshape(128, 1)
    xrootT = np.ascontiguousarray(x[root_index].T)

    in_maps = []
    for c in range(NC_):
        in_maps.append(dict(
            xcT=np.ascontiguousarray(x[c * RPC:(c + 1) * RPC].T),
            xrootT=xrootT,
            w1=w1, w2a=w2a, w2b=w2b, bias1=bias1, bias2=bias2,
            deg=np.ascontiguousarray(deg[c]),
            srcsY=np.ascontiguousarray(srcs16[c]), srcsZ=np.ascontiguousarray(srcsZ16[c]), drel=np.ascontiguousarray(drel[c]),
            brel=np.ascontiguousarray(brel[c]), browb=np.ascontiguousarray(browb[c]),
            rshb=np.ascontiguousarray(rshb[c]), cntf=cntf,
            iota_in=iota_in, iotac_in=iotac_in, fcw=np.ascontiguousarray(fcw), fcb=fcb,
        ))
    return TB, in_maps


def kernel(**inputs):
    from concourse.bass_utils import run_bass_kernel_spmd
    TB, in_maps = _prep(**inputs)
    if TB not in _cache:
        _cache[TB] = _build(TB)
    nc = _cache[TB]
    res = run_bass_kernel_spmd(nc, in_maps, list(range(NC_)))
    return res.results[0]["out"]


if __name__ == "__main__":
    import reference
    inputs = {k: np.asarray(v) for k, v in reference.setup_inputs().items()}
    got = kernel(**inputs)
    print(got[:4])
